# revision 1
# baseline (speedup 1.0000x reference)
"""EnVAE sampling kernel for 8x TRN2 NeuronCores.

Math (per group g, batch element b):
  Xg = X[:, g::8]                                     # (b, 128)
  h  = relu(Xg @ W1[g] + b1[g])                        # (b, 128)
  out= h @ W2[g] + b2[g]; means=out[:, :64]; lv=out[:, 64:]
  z  = means[b, idx] + eps * exp(0.5 * lv[b, idx])

Device computes (batch-sharded 8 ways, fp16 matmuls):
  zM[g,b] = (W2m[g]^T h)[idx[g,b], b]        (via onehot Hadamard + reduce-matmul)
  zX[g,b] = exp(0.5*L + 0.5*b2v[g])[idx[g,b], b]
Host finishes: z = zM + b2m[g, idx] + eps * zX
"""

import numpy as np
import ml_dtypes

import concourse.bass as bass
import concourse.bacc as bacc
import concourse.mybir as mybir
from concourse import tile
from concourse import bass_utils

OBS = 1024
LAT = 64
G = 8
GS = 128
HID = 128
BATCH = 65536
NCORES = 8
BPC = BATCH // NCORES        # 8192 batch rows per core
SC = 512                     # batch rows per superchunk
NPAIR = G // 2
BF16 = mybir.dt.float16  # fp16: same PE rate as bf16, 8x mantissa
F32 = mybir.dt.float32

# group n takes columns n, n+8, ... (round-robin)
GROUP_IDX = np.stack([np.arange(n, OBS, G) for n in range(G)])  # (g, gs)


def build_program(nsc: int, num_devices: int = NCORES):
    """Build the per-core bass program for nsc superchunks of SC batch rows."""
    B = nsc * SC
    nc = bacc.Bacc("TRN2", target_bir_lowering=False, debug=False,
                   num_devices=num_devices)

    QUAD = 4 if nsc % 4 == 0 else 1
    nquad = nsc // QUAD
    QW = QUAD * SC
    # DRAM inputs (per-core shard)
    # xt: quad-block-major [nquad, G, QW, GS] fp16
    xt = nc.dram_tensor("xt", [nquad, G, QW, GS], BF16, kind="ExternalInput").ap()
    # onehot, transposed per pair: [nquad, NPAIR, 128, QW] int8
    #   partitions 0:64   = onehot[g=2*pair]   (latent on partition)
    #   partitions 64:128 = onehot[g=2*pair+1]
    oh = nc.dram_tensor("oh", [nquad, NPAIR, 128, QW], mybir.dt.int8,
                        kind="ExternalInput").ap()
    w1 = nc.dram_tensor("w1", [G, GS, HID], BF16, kind="ExternalInput").ap()
    # w2 packed per pair: [NPAIR, 2(tensor: m/v), GS, 2(group), LAT] bf16
    w2m = nc.dram_tensor("w2m", [G, GS, LAT], BF16, kind="ExternalInput").ap()
    w2v = nc.dram_tensor("w2v", [G, GS, LAT], BF16, kind="ExternalInput").ap()
    b1 = nc.dram_tensor("b1", [G, GS], F32, kind="ExternalInput").ap()
    # hb2v[pair] = per-partition bias col for exp: [NPAIR, 128] f32
    hb2v = nc.dram_tensor("hb2v", [NPAIR, 128], F32, kind="ExternalInput").ap()
    # selector for the reduce matmul: [2, 128, 4] bf16
    sel = nc.dram_tensor("sel", [2, 128, 4], BF16, kind="ExternalInput").ap()
    # output: [128, nsc*NPAIR*16] f32; col = ((sc*NPAIR + pair)*4 + c)*4 + q
    zout = nc.dram_tensor("z", [128, nsc * NPAIR * 16], F32,
                          kind="ExternalOutput").ap()

    from contextlib import ExitStack
    with tile.TileContext(nc) as tc, ExitStack() as st:
        # --- resident constants ---
        cp = st.enter_context(tc.tile_pool(name="const", bufs=1))
        if True:
            w1_sb = cp.tile([GS, G, HID], BF16, tag="w1")
            nc.sync.dma_start(w1_sb[:], w1.rearrange("g k m -> k g m"))
            w2m_sb = cp.tile([GS, G, LAT], BF16, tag="w2m")
            nc.sync.dma_start(w2m_sb[:], w2m.rearrange("g k m -> k g m"))
            w2v_sb = cp.tile([GS, G, LAT], BF16, tag="w2v")
            nc.sync.dma_start(w2v_sb[:], w2v.rearrange("g k m -> k g m"))
            b1_sb = cp.tile([GS, G], F32, tag="b1")
            nc.sync.dma_start(b1_sb[:], b1.rearrange("g k -> k g"))
            hb2v_sb = cp.tile([128, NPAIR], F32, tag="hb2v")
            nc.sync.dma_start(hb2v_sb[:], hb2v.rearrange("p k -> k p"))
            sel_sb = cp.tile([128, 2, 4], BF16, tag="sel")
            nc.sync.dma_start(sel_sb[:], sel.rearrange("t k f -> k t f"))

            # persistent z staging + z psum banks
            zpool = st.enter_context(tc.tile_pool(name="zp", bufs=1, space="PSUM"))
            xpool = st.enter_context(tc.tile_pool(name="xt", bufs=16))
            ohpool = st.enter_context(tc.tile_pool(name="ohp", bufs=8))
            hpsum = st.enter_context(tc.tile_pool(name="hps", bufs=3, space="PSUM"))
            hpool = st.enter_context(tc.tile_pool(name="hsb", bufs=8))
            mvpsum = st.enter_context(tc.tile_pool(name="mvps", bufs=2, space="PSUM"))
            ppool = st.enter_context(tc.tile_pool(name="prod", bufs=8))
            zsbp = st.enter_context(tc.tile_pool(name="zsb", bufs=1))

            if True:
                ZCOLS = 16  # cols per (pair, sc) in the z psum tile: 4 chunks x 4 q
                # one z psum tile per 32 (pair,sc) instances (512 cols each)
                nzt = (nsc * NPAIR + 31) // 32
                ztiles = [zpool.tile([128, 512], F32, name=f"zt{i}", tag="z")
                          for i in range(nzt)]
                zsb = zsbp.tile([128, nsc * NPAIR * 16], F32, tag="zstage")

                pending = []
                stage2 = []
                drained = set()

                def _emit_stage2(item):
                    inst, bM, bX, oht_, pr = item
                    prodM = ppool.tile([128, SC], BF16, name="prodM",
                                       tag="prodM")
                    nc.vector.tensor_tensor(prodM[:], bM[:], oht_,
                                            mybir.AluOpType.mult)
                    xsb = ppool.tile([128, SC], BF16, name="xsb", tag="xsb")
                    nc.scalar.activation(
                        xsb[:], bX[:],
                        mybir.ActivationFunctionType.Exp,
                        bias=hb2v_sb[:, pr:pr + 1], scale=0.5)
                    prodX = ppool.tile([128, SC], BF16, name="prodX",
                                       tag="prodX")
                    nc.gpsimd.tensor_tensor(prodX[:], xsb[:], oht_,
                                            mybir.AluOpType.mult)
                    pending.append((inst, prodM, prodX))

                def _drain(done_tile_idx):
                    # after the last zred of a z tile, copy it out so the
                    # single psum slot can recycle
                    if done_tile_idx is not None:
                        i = done_tile_idx
                        w = min(512, nsc * NPAIR * 16 - i * 512)
                        nc.scalar.copy(zsb[:, i * 512:i * 512 + w],
                                       ztiles[i][:, :w])
                        drained.add(i)

                def _emit_zred(item):
                    inst, pM, pX = item
                    zt = ztiles[inst // 32]
                    zoff = (inst % 32) * ZCOLS
                    for c in range(4):
                        zslice = zt[:, zoff + 4 * c: zoff + 4 * c + 4]
                        nc.tensor.matmul(
                            zslice, pM[:, 128 * c:128 * c + 128],
                            sel_sb[:, 0], start=True, stop=False,
                            skip_group_check=True)
                        nc.tensor.matmul(
                            zslice, pX[:, 128 * c:128 * c + 128],
                            sel_sb[:, 1], start=False, stop=True,
                            skip_group_check=True)

                for quad in range(nquad):
                    # --- bulk loads: XgT for all 8 groups, oh for all pairs
                    xg = [xpool.tile([GS, QW], BF16, name=f"xg{g}", tag="xg")
                          for g in range(G)]
                    for g in range(G):
                        nc.sync.dma_start(xg[g][:], xt[quad, g], transpose=True)
                    ohq = [ohpool.tile([128, QW], mybir.dt.int8,
                                        name=f"oh{p}", tag="oh")
                           for p in range(NPAIR)]
                    for p in range(NPAIR):
                        nc.sync.dma_start(ohq[p][:], oh[quad, p])

                    for scq in range(QUAD):
                        sc = quad * QUAD + scq
                        so = scq * SC
                        for pair in range(NPAIR):
                            g0, g1 = 2 * pair, 2 * pair + 1
                            oht = ohq[pair][:, so:so + SC]

                            # --- mm1 + relu per group (relu alternates ACT/DVE)
                            hsb = [hpool.tile([HID, SC], BF16, name=f"hsb{_i}",
                                              tag="h") for _i in range(2)]
                            for i, g in enumerate((g0, g1)):
                                hp = hpsum.tile([HID, SC], F32, tag="hpsum")
                                nc.tensor.matmul(hp[:], w1_sb[:, g],
                                                 xg[g][:, so:so + SC],
                                                 start=True, stop=True)
                                if i == 0:
                                    # g0 relu on ACT, g1 on DVE: they run
                                    # concurrently, unblocking mm2 sooner
                                    nc.scalar.activation(
                                        hsb[i][:], hp[:],
                                        mybir.ActivationFunctionType.Relu,
                                        bias=b1_sb[:, g:g + 1], scale=1.0)
                                else:
                                    nc.vector.tensor_scalar(
                                        hsb[i][:], hp[:],
                                        b1_sb[:, g:g + 1], 0.0,
                                        mybir.AluOpType.add,
                                        mybir.AluOpType.max)

                            # --- mm2: col-packed pairs (means first) ---
                            bankM = mvpsum.tile([128, SC], F32, tag="bankM")
                            bankX = mvpsum.tile([128, SC], F32, tag="bankX")
                            for i, g in enumerate((g0, g1)):
                                nc.tensor.matmul(bankM[64 * i:64 * i + 64, :],
                                                 w2m_sb[:, g], hsb[i][:],
                                                 start=True, stop=True,
                                                 tile_position=(0, 64 * i))
                            for i, g in enumerate((g0, g1)):
                                nc.tensor.matmul(bankX[64 * i:64 * i + 64, :],
                                                 w2v_sb[:, g], hsb[i][:],
                                                 start=True, stop=True,
                                                 tile_position=(0, 64 * i))

                            # --- stage-2 (Hadamard + exp) for the PREVIOUS
                            # iteration: keeps every engine FIFO free of
                            # head-of-line waits on just-issued matmuls
                            inst = sc * NPAIR + pair
                            stage2.append((inst, bankM, bankX, oht, pair))
                            if len(stage2) > 1:
                                _emit_stage2(stage2.pop(0))
                            if len(pending) > 2:
                                _drain(_emit_zred(pending.pop(0)))

                for item in stage2:
                    _emit_stage2(item)
                for item in pending:
                    _drain(_emit_zred(item))
                for i, zt in enumerate(ztiles):
                    if i not in drained:
                        w = min(512, nsc * NPAIR * 16 - i * 512)
                        nc.vector.tensor_copy(zsb[:, i * 512:i * 512 + w],
                                              zt[:, :w])
                nc.sync.dma_start(zout[:], zsb[:])

    nc.compile()
    return nc


# ---------------------------------------------------------------- host side --

def _prep_host(X, eps, W1, b1, W2, b2, indices, nsc=BPC // SC, ncores=NCORES):
    """Build per-core input dicts + closures for unscrambling."""
    B = nsc * SC
    bf = np.float16
    # X: permute columns group-major, cast bf16, block layout [nsc, G, SC, GS]
    Xp = np.ascontiguousarray(X[:, GROUP_IDX.reshape(-1)]).astype(bf)  # (BATCH, 1024)
    W1b = W1.astype(bf)                              # (g, gs, hid)
    W2m = np.ascontiguousarray(W2[:, :, :LAT]).astype(bf)
    W2v = np.ascontiguousarray(W2[:, :, LAT:]).astype(bf)
    b1f = b1.astype(np.float32)
    hb2v = np.zeros((NPAIR, 128), np.float32)
    for p in range(NPAIR):
        hb2v[p, :64] = 0.5 * b2[2 * p, LAT:]
        hb2v[p, 64:] = 0.5 * b2[2 * p + 1, LAT:]
    selm = np.zeros((2, 128, 4), np.float32)
    selm[0, :64, 0] = 1.0   # zM g0
    selm[0, 64:, 1] = 1.0   # zM g1
    selm[1, :64, 2] = 1.0   # zX g0
    selm[1, 64:, 3] = 1.0   # zX g1
    selb = selm.astype(bf)

    QUAD = 4 if nsc % 4 == 0 else 1
    nquad = nsc // QUAD
    QW = QUAD * SC
    in_maps = []
    for core in range(ncores):
        lo = core * B
        Xc = Xp[lo:lo + B].reshape(nquad, QW, G, GS)
        xt = np.ascontiguousarray(Xc.transpose(0, 2, 1, 3))      # (nq,G,QW,GS)
        idxc = indices[:, lo:lo + B]                             # (G, B)
        ohc = np.zeros((nquad, NPAIR, 128, QW), np.float32)
        ar = np.arange(LAT)
        for p in range(NPAIR):
            for i, g in enumerate((2 * p, 2 * p + 1)):
                ii = idxc[g].reshape(nquad, QW)                  # (nq, QW)
                m = (ii[:, None, :] == ar[None, :, None])        # (nq, 64, QW)
                ohc[:, p, 64 * i:64 * i + 64, :] = m
        in_maps.append({
            "xt": xt, "oh": ohc.astype(np.int8), "w1": W1b, "w2m": W2m, "w2v": W2v,
            "b1": b1f, "hb2v": hb2v, "sel": selb,
        })
    return in_maps


def _unscramble(zdev, nsc):
    """zdev: (128, nsc*NPAIR*16) f32 -> zM, zX each (G, nsc*SC)."""
    B = nsc * SC
    zr = zdev.reshape(128, nsc, NPAIR, 4, 4)       # p, sc, pair, c, q
    zM = np.zeros((G, B), np.float32)
    zX = np.zeros((G, B), np.float32)
    for pair in range(NPAIR):
        for q, (dst, g) in enumerate(((zM, 2 * pair), (zM, 2 * pair + 1),
                                      (zX, 2 * pair), (zX, 2 * pair + 1))):
            blk = zr[:, :, pair, :, q]             # (128, nsc, 4)
            dst[g] = blk.transpose(1, 2, 0).reshape(B)
    return zM, zX


_NC_CACHE = {}


def kernel(X, eps, W1, b1, W2, b2, indices):
    nsc = BPC // SC
    key = (nsc, NCORES)
    if key not in _NC_CACHE:
        _NC_CACHE[key] = build_program(nsc, NCORES)
    nc = _NC_CACHE[key]
    in_maps = _prep_host(X, eps, W1, b1, W2, b2, indices)
    res = bass_utils.run_bass_kernel_spmd(nc, in_maps, core_ids=list(range(NCORES)))

    z = np.zeros((G, BATCH), np.float32)
    B = nsc * SC
    for core in range(NCORES):
        lo = core * B
        zM, zX = _unscramble(res.results[core]["z"], nsc)
        idxc = indices[:, lo:lo + B]
        b2m_sel = np.take_along_axis(b2[:, :LAT], idxc, axis=1)
        z[:, lo:lo + B] = zM + b2m_sel + eps[:, lo:lo + B] * zX
    return z.astype(np.float32)



# revision 6
# speedup vs baseline: 1.7420x; 1.7420x over previous
"""EnVAE sampling kernel for 8x TRN2 NeuronCores — sorted-batch fused-selection design.

Math (per group g, batch element b):
  Xg = X[:, g::8]                                      # (b, 128)
  h  = relu(Xg @ W1[g] + b1[g])                        # (b, 128)
  out= h @ W2[g] + b2[g]; means=out[:, :64]; lv=out[:, 64:]
  z  = means[b, idx] + eps * exp(0.5 * lv[b, idx])

Key trick: each group g reads a DISJOINT column slice of X, so the host can
reorder each group's batch independently — sort by idx[g]. Then within any
128-column chunk of the sorted batch, at most ~3 distinct latents appear, and
mm2 + latent selection fuse into <=3 tiny matmuls per chunk:
  stationary = h-chunk [128 hid, 128 batch] (SBUF)
  moving     = the 2 columns of W2 for that run's latent (mean, logvar)
  out        = [128 batch, 2] cols of the per-group z psum tile
No onehot, no Hadamard, no on-device exp. Host finishes:
  z = zm + b2m[g, idx] + eps * exp(0.5*(zv + b2v[g, idx]))

Device mm1 runs fp8e4m3 in DoubleRow perf mode (2 contraction slots per
partition, X packed [64, 2, b]); W1 is pre-scaled by 16 to stay out of fp8
denormals and W2 pre-divided by 16 to compensate (relu(a*x) = a*relu(x)).
"""

import numpy as np
import ml_dtypes

import concourse.bass as bass
import concourse.bacc as bacc
import concourse.mybir as mybir
from concourse import tile
from concourse import bass_utils

OBS = 1024
LAT = 64
G = 8
GS = 128
HID = 128
BATCH = 65536
NCORES = 8
BPC = BATCH // NCORES        # 8192 batch rows per core
SC = 1024                    # batch rows per superchunk (relu granularity)
NSC = BPC // SC              # 8
CHUNK = 128                  # batch rows per mm2sel chunk (PE stationary width)
NCH = BPC // CHUNK           # 64 chunks per (group, core)
SEGS = 3                     # padded segments per chunk (fixed for SPMD)
ZC = NCH * SEGS * 2          # z cols per group = 384
W1SCALE = 16.0

FP8 = mybir.dt.float8e4
BF16 = mybir.dt.bfloat16
F32 = mybir.dt.float32
NP_FP8 = ml_dtypes.float8_e4m3
NP_BF16 = ml_dtypes.bfloat16

# group n takes columns n, n+8, ... (round-robin)
GROUP_IDX = np.stack([np.arange(n, OBS, G) for n in range(G)])  # (g, gs)


def build_program(num_devices: int = NCORES):
    """Per-core bass program (SPMD: identical across cores; per-core data
    differences live in xt / w2sel)."""
    nc = bacc.Bacc("TRN2", target_bir_lowering=False, debug=False,
                   num_devices=num_devices)

    # xt[g, sc, p, i, b] = Xg_sorted[sc*SC + b, p + 64*i]  (fp8)
    xt = nc.dram_tensor("xt", [G, NSC, 64, 2, SC], FP8, kind="ExternalInput").ap()
    # w1[p, g, i, m] = 16 * W1[g, p + 64*i, m]  (fp8)
    w1 = nc.dram_tensor("w1", [64, G, 2, HID], FP8, kind="ExternalInput").ap()
    # w2sel[g, k, ch, s, j] = W2[g, k, l(g,ch,s) + 64*j] / 16  (bf16)
    w2sel = nc.dram_tensor("w2sel", [G, HID, NCH, SEGS, 2], BF16,
                           kind="ExternalInput").ap()
    # b1s[k, g] = 16 * b1[g, k]
    b1 = nc.dram_tensor("b1", [HID, G], F32, kind="ExternalInput").ap()
    # zout[g][row, (ch*SEGS+s)*2 + j]: j=0 -> zm, j=1 -> zv  (bf16)
    zout = nc.dram_tensor("z", [G, CHUNK, ZC], BF16, kind="ExternalOutput").ap()

    # --- static engine load balancer for the vector ops -------------------
    # op cost model (ns) for [*, n]-col ops per engine; greedy least-loaded
    eng_time = {"act": 0.0, "dve": 0.0}

    def relu_cost(e, n):
        if e == "act":
            return n * 0.833 + 185.0
        return n * 1.042 + 125.0

    def pick_engine(n):
        e = min(eng_time, key=lambda k: eng_time[k] + relu_cost(k, n))
        eng_time[e] += relu_cost(e, n)
        return e

    from contextlib import ExitStack
    with tile.TileContext(nc) as tc, ExitStack() as st:
        cp = st.enter_context(tc.tile_pool(name="const", bufs=1))
        w1_sb = cp.tile([64, G, 2, HID], FP8, tag="w1")
        nc.sync.dma_start(w1_sb[:], w1)
        b1_sb = cp.tile([HID, G], F32, tag="b1")
        nc.sync.dma_start(b1_sb[:], b1)

        xpool = st.enter_context(tc.tile_pool(name="xg", bufs=6))
        wspool = st.enter_context(tc.tile_pool(name="ws", bufs=2))
        hpool = st.enter_context(tc.tile_pool(name="hsb", bufs=5))
        zspool = st.enter_context(tc.tile_pool(name="zsb", bufs=2))
        hpsum = st.enter_context(tc.tile_pool(name="hp", bufs=3, space="PSUM"))
        zpsum = st.enter_context(tc.tile_pool(name="zt", bufs=2, space="PSUM"))

        relu_fns = {
            "act": lambda o, i, b: nc.scalar.activation(
                o, i, mybir.ActivationFunctionType.Relu, bias=b, scale=1.0),
            "dve": lambda o, i, b: nc.vector.tensor_scalar(
                o, i, b, 0.0, mybir.AluOpType.add, mybir.AluOpType.max),
        }
        copy_fns = {
            "act": nc.scalar.copy,
            "dve": nc.vector.tensor_copy,
        }


        # software-pipelined emission: PE sel-matmuls run one instance behind
        pending = []            # (g, sc, hsb, zt)
        gdone = []              # (g, zt) awaiting drain after last sel emitted

        def emit_sel(item):
            g, sc, hsb, zt = item
            for cc in range(SC // CHUNK):
                ch = sc * (SC // CHUNK) + cc
                for s in range(SEGS):
                    col = (ch * SEGS + s) * 2
                    nc.tensor.matmul(
                        zt[:, col:col + 2],
                        hsb[:, CHUNK * cc:CHUNK * (cc + 1)],
                        wsel_tiles[g][:, ch, s],
                        start=True, stop=True, skip_group_check=True)
            if sc == NSC - 1:
                gdone.append((g, zt))

        def emit_drain():
            g, zt = gdone.pop(0)
            e = pick_engine(ZC)
            zsb = zspool.tile([CHUNK, ZC], BF16, name=f"zsb{g}", tag="zsb")
            copy_fns[e](zsb[:], zt[:, :ZC])
            nc.scalar.dma_start(zout[g], zsb[:])

        wsel_tiles = {}
        zts = {}
        for g in range(G):
            wsel = wspool.tile([HID, NCH, SEGS, 2], BF16, name=f"ws{g}",
                               tag="wsel")
            nc.sync.dma_start(wsel[:], w2sel[g])
            wsel_tiles[g] = wsel
            zt = zpsum.tile([CHUNK, 512], F32, name=f"zt{g}", tag="zt")
            zts[g] = zt
            for sc in range(NSC):
                xg = xpool.tile([64, 2, SC], FP8, name=f"xg{g}_{sc}", tag="xg")
                nc.sync.dma_start(xg[:], xt[g, sc])
                hp = hpsum.tile([HID, SC], F32, tag="hp")
                for half in range(2):
                    nc.tensor.matmul(
                        hp[:, 512 * half:512 * (half + 1)],
                        w1_sb[:, g],
                        xg[:, :, 512 * half:512 * (half + 1)],
                        start=True, stop=True,
                        perf_mode=mybir.MatmulPerfMode.DoubleRow)
                hsb = hpool.tile([HID, SC], BF16, tag="hsb")
                e = pick_engine(SC)
                relu_fns[e](hsb[:], hp[:], b1_sb[:, g:g + 1])

                pending.append((g, sc, hsb, zt))
                if len(pending) > 1:
                    emit_sel(pending.pop(0))
                if gdone:
                    emit_drain()
        while pending:
            emit_sel(pending.pop(0))
        while gdone:
            emit_drain()

    nc.compile()
    return nc


# ---------------------------------------------------------------- host side --

def _prep_host(X, eps, W1, b1, W2, b2, indices, ncores=NCORES):
    """Per-core input dicts + metadata for unscrambling."""
    W1p = np.ascontiguousarray(
        (W1 * W1SCALE).reshape(G, 2, 64, HID).transpose(2, 0, 1, 3)
    ).astype(NP_FP8)                                   # (64, G, 2, HID)
    b1s = np.ascontiguousarray((W1SCALE * b1).T).astype(np.float32)  # (HID, G)
    W2s = (W2 / W1SCALE).astype(np.float32)            # (G, HID, 128)

    in_maps = []
    metas = []
    for core in range(ncores):
        lo = core * BPC
        xt = np.empty((G, NSC, 64, 2, SC), NP_FP8)
        w2sel = np.empty((G, HID, NCH, SEGS, 2), NP_BF16)
        meta = []
        for g in range(G):
            idxg = indices[g, lo:lo + BPC]
            order = np.argsort(idxg, kind="stable")    # sorted batch positions
            slat = idxg[order]                         # (BPC,) sorted latents
            Xg = X[lo + order][:, GROUP_IDX[g]].astype(NP_FP8)  # (BPC, 128)
            # pack [sc, p, i, b]: col k = p + 64*i
            xt[g] = (Xg.reshape(NSC, SC, 2, 64)
                     .transpose(0, 3, 2, 1))           # (NSC, 64, 2, SC)
            # segments: distinct latents per 128-chunk, padded to SEGS
            lat_ch = slat.reshape(NCH, CHUNK)
            seg_lat = np.zeros((NCH, SEGS), np.int64)
            seg_of_pos = np.empty(BPC, np.int64)
            for ch in range(NCH):
                uniq, inv = np.unique(lat_ch[ch], return_inverse=True)
                ns = len(uniq)
                assert ns <= SEGS, f"chunk needs {ns} segments > SEGS={SEGS}"
                seg_lat[ch, :ns] = uniq
                seg_of_pos[ch * CHUNK:(ch + 1) * CHUNK] = inv
            # w2sel[k, ch, s, j] = W2s[g][k, seg_lat[ch,s] + 64*j]
            cols = (seg_lat[None, :, :, None] +
                    64 * np.arange(2)[None, None, None, :])  # (1, NCH, SEGS, 2)
            w2sel[g] = W2s[g][:, cols[0]].astype(NP_BF16)
            meta.append((order, slat, seg_of_pos))
        in_maps.append({"xt": xt, "w1": W1p, "w2sel": w2sel, "b1": b1s})
        metas.append(meta)
    return in_maps, metas


def _finish_host(zdev, meta, eps_c, b2):
    """zdev: (G, CHUNK, ZC) f32; returns z (G, BPC) in original batch order."""
    z = np.empty((G, BPC), np.float32)
    pos = np.arange(BPC)
    rows = pos % CHUNK
    ch = pos // CHUNK
    for g in range(G):
        order, slat, seg_of_pos = meta[g]
        col = (ch * SEGS + seg_of_pos) * 2
        zm = zdev[g][rows, col]
        zv = zdev[g][rows, col + 1]
        zs = (zm + b2[g, slat] +
              eps_c[g, order] * np.exp(0.5 * (zv + b2[g, LAT + slat])))
        z[g, order] = zs
    return z


_NC_CACHE = {}


def kernel(X, eps, W1, b1, W2, b2, indices):
    if "nc" not in _NC_CACHE:
        _NC_CACHE["nc"] = build_program(NCORES)
    nc = _NC_CACHE["nc"]
    in_maps, metas = _prep_host(X, eps, W1, b1, W2, b2, indices)
    res = bass_utils.run_bass_kernel_spmd(nc, in_maps,
                                          core_ids=list(range(NCORES)))
    z = np.zeros((G, BATCH), np.float32)
    for core in range(NCORES):
        lo = core * BPC
        zdev = np.asarray(res.results[core]["z"]).astype(np.float32)
        z[:, lo:lo + BPC] = _finish_host(zdev, metas[core],
                                         eps[:, lo:lo + BPC], b2)
    return z.astype(np.float32)


# revision 13
# speedup vs baseline: 1.8089x; 1.0384x over previous
"""EnVAE sampling kernel for 8x TRN2 NeuronCores — sorted-batch fused-selection design.

Math (per group g, batch element b):
  Xg = X[:, g::8]                                      # (b, 128)
  h  = relu(Xg @ W1[g] + b1[g])                        # (b, 128)
  out= h @ W2[g] + b2[g]; means=out[:, :64]; lv=out[:, 64:]
  z  = means[b, idx] + eps * exp(0.5 * lv[b, idx])

Key trick: each group g reads a DISJOINT column slice of X, so the host can
reorder each group's batch independently — sort by idx[g]. Then within any
128-column chunk of the sorted batch, at most ~3 distinct latents appear, and
mm2 + latent selection fuse into <=3 tiny matmuls per chunk:
  stationary = h-chunk [128 hid, 128 batch] (SBUF)
  moving     = the 2 columns of W2 for that run's latent (mean, logvar)
  out        = [128 batch, 2] cols of the per-group z psum tile
No onehot, no Hadamard, no on-device exp. Host finishes:
  z = zm + b2m[g, idx] + eps * exp(0.5*(zv + b2v[g, idx]))

Device mm1 runs fp8e4m3 in DoubleRow perf mode (2 contraction slots per
partition, X packed [64, 2, b]); W1 is pre-scaled by 16 to stay out of fp8
denormals and W2 pre-divided by 16 to compensate (relu(a*x) = a*relu(x)).
"""

import numpy as np
import ml_dtypes

import concourse.bass as bass
import concourse.bacc as bacc
import concourse.mybir as mybir
from concourse import tile
from concourse import bass_utils

OBS = 1024
LAT = 64
G = 8
GS = 128
HID = 128
BATCH = 65536
NCORES = 8
BPC = BATCH // NCORES        # 8192 batch rows per core
SC = 1024                    # batch rows per superchunk (relu granularity)
NSC = BPC // SC              # 8
CHUNK = 128                  # batch rows per mm2sel chunk (PE stationary width)
NCH = BPC // CHUNK           # 64 chunks per (group, core)
SEGS = 3                     # padded segments per chunk (fixed for SPMD)
ZC = NCH * SEGS * 2          # z cols per group = 384
W1SCALE = 16.0

FP8 = mybir.dt.float8e4
BF16 = mybir.dt.bfloat16
F32 = mybir.dt.float32
NP_FP8 = ml_dtypes.float8_e4m3
NP_BF16 = ml_dtypes.bfloat16

# group n takes columns n, n+8, ... (round-robin)
GROUP_IDX = np.stack([np.arange(n, OBS, G) for n in range(G)])  # (g, gs)


def build_program(num_devices: int = NCORES):
    """Per-core bass program (SPMD: identical across cores; per-core data
    differences live in xt / w2sel)."""
    nc = bacc.Bacc("TRN2", target_bir_lowering=False, debug=False,
                   num_devices=num_devices)

    # xt[g, p, sc, i, b] = Xg_sorted[sc*SC + b, p + 64*i]  (fp8)
    xt = nc.dram_tensor("xt", [G, 64, NSC, 2, SC], FP8, kind="ExternalInput").ap()
    # w1[p, g, i, m] = 16 * W1[g, p + 64*i, m]  (fp8)
    w1 = nc.dram_tensor("w1", [64, G, 2, HID], FP8, kind="ExternalInput").ap()
    # w2sel[k, g, ch, s, j] = W2[g, k, l(g,ch,s) + 64*j] / 16  (bf16)
    w2sel = nc.dram_tensor("w2sel", [HID, G, NCH, SEGS, 2], BF16,
                           kind="ExternalInput").ap()
    # b1s[k, g] = 16 * b1[g, k]
    b1 = nc.dram_tensor("b1", [HID, G], F32, kind="ExternalInput").ap()
    # zout[g][row, (ch*SEGS+s)*2 + j]: j=0 -> zm, j=1 -> zv  (bf16)
    zout = nc.dram_tensor("z", [G, CHUNK, ZC], BF16, kind="ExternalOutput").ap()

    # --- static engine load balancer for the vector ops -------------------
    # op cost model (ns) for [*, n]-col ops per engine; greedy least-loaded
    eng_time = {"act": 0.0, "dve": 0.0}

    def relu_cost(e, n):
        if e == "act":
            return n * 0.833 + 185.0
        return n * 1.042 + 125.0

    def pick_engine(n):
        e = min(eng_time, key=lambda k: eng_time[k] + relu_cost(k, n))
        eng_time[e] += relu_cost(e, n)
        return e

    from contextlib import ExitStack
    with tile.TileContext(nc) as tc, ExitStack() as st:
        cp = st.enter_context(tc.tile_pool(name="const", bufs=1))
        w1_sb = cp.tile([64, G, 2, HID], FP8, tag="w1")
        nc.sync.dma_start(w1_sb[:], w1)
        b1_sb = cp.tile([HID, G], F32, tag="b1")
        nc.sync.dma_start(b1_sb[:], b1)

        xpool = st.enter_context(tc.tile_pool(name="xg", bufs=3))
        wspool = st.enter_context(tc.tile_pool(name="ws", bufs=2))
        hpool = st.enter_context(tc.tile_pool(name="hsb", bufs=5))
        zspool = st.enter_context(tc.tile_pool(name="zsb", bufs=2))
        hpsum = st.enter_context(tc.tile_pool(name="hp", bufs=3, space="PSUM"))
        zpsum = st.enter_context(tc.tile_pool(name="zt", bufs=2, space="PSUM"))

        relu_fns = {
            "act": lambda o, i, b: nc.scalar.activation(
                o, i, mybir.ActivationFunctionType.Relu, bias=b, scale=1.0),
            "dve": lambda o, i, b: nc.vector.tensor_scalar(
                o, i, b, 0.0, mybir.AluOpType.add, mybir.AluOpType.max),
        }
        copy_fns = {
            "act": nc.scalar.copy,
            "dve": nc.vector.tensor_copy,
        }


        # software-pipelined emission: PE sel-matmuls run one instance behind
        pending = []            # (g, sc, hsb, zt)
        gdone = []              # (g, zt) awaiting drain after last sel emitted

        def emit_sel(item):
            g, sc, hsb, zt = item
            for cc in range(SC // CHUNK):
                ch = sc * (SC // CHUNK) + cc
                for s in range(SEGS):
                    col = (ch * SEGS + s) * 2
                    nc.tensor.matmul(
                        zt[:, col:col + 2],
                        hsb[:, CHUNK * cc:CHUNK * (cc + 1)],
                        wsel_tiles[g][:, g % 4, ch, s],
                        start=True, stop=True, skip_group_check=True)
            if sc == NSC - 1:
                gdone.append((g, zt))

        def emit_drain():
            g, zt = gdone.pop(0)
            e = pick_engine(ZC)
            zsb = zspool.tile([CHUNK, ZC], BF16, name=f"zsb{g}", tag="zsb")
            copy_fns[e](zsb[:], zt[:, :ZC])
            # SWDGE path: Pool engine is otherwise idle and bypasses the
            # shared HWDGE slot.
            nc.gpsimd.dma_start(zout[g], zsb[:])

        wsel_tiles = {}
        for g in range(G):
            # prefetch xt for group g as two half-DMAs (amortizes the ~625ns
            # HWDGE fixed cost while keeping startup latency low)
            xg = xpool.tile([64, NSC, 2, SC], FP8, name=f"xg{g}", tag="xg")
            half_sc = NSC // 2
            for h in range(2):
                nc.sync.dma_start(xg[:, h * half_sc:(h + 1) * half_sc],
                                  xt[g, :, h * half_sc:(h + 1) * half_sc])
            if g % 4 == 0:
                wsel = wspool.tile([HID, 4, NCH, SEGS, 2], BF16,
                                   name=f"ws{g // 4}", tag="wsel")
                nc.sync.dma_start(wsel[:], w2sel[:, g:g + 4])
                for gg in range(g, g + 4):
                    wsel_tiles[gg] = wsel
            zt = zpsum.tile([CHUNK, 512], F32, name=f"zt{g}", tag="zt")
            for sc in range(NSC):
                hp = hpsum.tile([HID, SC], F32, tag="hp")
                for half in range(2):
                    nc.tensor.matmul(
                        hp[:, 512 * half:512 * (half + 1)],
                        w1_sb[:, g],
                        xg[:, sc, :, 512 * half:512 * (half + 1)],
                        start=True, stop=True,
                        perf_mode=mybir.MatmulPerfMode.DoubleRow)
                hsb = hpool.tile([HID, SC], BF16, tag="hsb")
                e = pick_engine(SC)
                relu_fns[e](hsb[:], hp[:], b1_sb[:, g:g + 1])

                pending.append((g, sc, hsb, zt))
                if len(pending) > 1:
                    emit_sel(pending.pop(0))
                if gdone:
                    emit_drain()
        while pending:
            emit_sel(pending.pop(0))
        while gdone:
            emit_drain()

    nc.compile()
    return nc


# ---------------------------------------------------------------- host side --

def _prep_host(X, eps, W1, b1, W2, b2, indices, ncores=NCORES):
    """Per-core input dicts + metadata for unscrambling."""
    W1p = np.ascontiguousarray(
        (W1 * W1SCALE).reshape(G, 2, 64, HID).transpose(2, 0, 1, 3)
    ).astype(NP_FP8)                                   # (64, G, 2, HID)
    b1s = np.ascontiguousarray((W1SCALE * b1).T).astype(np.float32)  # (HID, G)
    W2s = (W2 / W1SCALE).astype(np.float32)            # (G, HID, 128)

    in_maps = []
    metas = []
    for core in range(ncores):
        lo = core * BPC
        xt = np.empty((G, 64, NSC, 2, SC), NP_FP8)
        w2sel = np.empty((HID, G, NCH, SEGS, 2), NP_BF16)
        meta = []
        for g in range(G):
            idxg = indices[g, lo:lo + BPC]
            order = np.argsort(idxg, kind="stable")    # sorted batch positions
            slat = idxg[order]                         # (BPC,) sorted latents
            Xg = X[lo + order][:, GROUP_IDX[g]].astype(NP_FP8)  # (BPC, 128)
            # pack [p, sc, i, b]: col k = p + 64*i
            xt[g] = (Xg.reshape(NSC, SC, 2, 64)
                     .transpose(3, 0, 2, 1))           # (64, NSC, 2, SC)
            # segments: distinct latents per 128-chunk, padded to SEGS
            lat_ch = slat.reshape(NCH, CHUNK)
            seg_lat = np.zeros((NCH, SEGS), np.int64)
            seg_of_pos = np.empty(BPC, np.int64)
            for ch in range(NCH):
                uniq, inv = np.unique(lat_ch[ch], return_inverse=True)
                ns = len(uniq)
                assert ns <= SEGS, f"chunk needs {ns} segments > SEGS={SEGS}"
                seg_lat[ch, :ns] = uniq
                seg_of_pos[ch * CHUNK:(ch + 1) * CHUNK] = inv
            # w2sel[k, ch, s, j] = W2s[g][k, seg_lat[ch,s] + 64*j]
            cols = (seg_lat[None, :, :, None] +
                    64 * np.arange(2)[None, None, None, :])  # (1, NCH, SEGS, 2)
            w2sel[:, g] = W2s[g][:, cols[0]].astype(NP_BF16)
            meta.append((order, slat, seg_of_pos))
        in_maps.append({"xt": xt, "w1": W1p, "w2sel": w2sel, "b1": b1s})
        metas.append(meta)
    return in_maps, metas


def _finish_host(zdev, meta, eps_c, b2):
    """zdev: (G, CHUNK, ZC) f32; returns z (G, BPC) in original batch order."""
    z = np.empty((G, BPC), np.float32)
    pos = np.arange(BPC)
    rows = pos % CHUNK
    ch = pos // CHUNK
    for g in range(G):
        order, slat, seg_of_pos = meta[g]
        col = (ch * SEGS + seg_of_pos) * 2
        zm = zdev[g][rows, col]
        zv = zdev[g][rows, col + 1]
        zs = (zm + b2[g, slat] +
              eps_c[g, order] * np.exp(0.5 * (zv + b2[g, LAT + slat])))
        z[g, order] = zs
    return z


_NC_CACHE = {}


def kernel(X, eps, W1, b1, W2, b2, indices):
    if "nc" not in _NC_CACHE:
        _NC_CACHE["nc"] = build_program(NCORES)
    nc = _NC_CACHE["nc"]
    in_maps, metas = _prep_host(X, eps, W1, b1, W2, b2, indices)
    res = bass_utils.run_bass_kernel_spmd(nc, in_maps,
                                          core_ids=list(range(NCORES)))
    z = np.zeros((G, BATCH), np.float32)
    for core in range(NCORES):
        lo = core * BPC
        zdev = np.asarray(res.results[core]["z"]).astype(np.float32)
        z[:, lo:lo + BPC] = _finish_host(zdev, metas[core],
                                         eps[:, lo:lo + BPC], b2)
    return z.astype(np.float32)


# revision 26
# speedup vs baseline: 2.0567x; 1.1370x over previous
"""EnVAE sampling kernel for 8x TRN2 NeuronCores — sorted-batch fused-selection design.

Math (per group g, batch element b):
  Xg = X[:, g::8]                                      # (b, 128)
  h  = relu(Xg @ W1[g] + b1[g])                        # (b, 128)
  out= h @ W2[g] + b2[g]; means=out[:, :64]; lv=out[:, 64:]
  z  = means[b, idx] + eps * exp(0.5 * lv[b, idx])

Key trick: each group g reads a DISJOINT column slice of X, so the host can
reorder each group's batch independently — sort by idx[g]. Then within any
128-column chunk of the sorted batch, at most ~3 distinct latents appear, and
mm2 + latent selection fuse into <=3 tiny matmuls per chunk:
  stationary = h-chunk [128 hid, 128 batch] (SBUF)
  moving     = the 2 columns of W2 for that run's latent (mean, logvar)
  out        = [128 batch, 2] cols of the per-group z psum tile
No onehot, no Hadamard, no on-device exp. Host finishes:
  z = zm + b2m[g, idx] + eps * exp(0.5*(zv + b2v[g, idx]))

Device mm1 runs fp8e4m3 in DoubleRow perf mode (2 contraction slots per
partition, X packed [64, 2, b]); W1 is pre-scaled by 16 to stay out of fp8
denormals and W2 pre-divided by 16 to compensate (relu(a*x) = a*relu(x)).
"""

import numpy as np
import ml_dtypes

import concourse.bass as bass
import concourse.bacc as bacc
import concourse.mybir as mybir
from concourse import tile
from concourse import bass_utils

OBS = 1024
LAT = 64
G = 8
GS = 128
HID = 128
BATCH = 65536
NCORES = 8
BPC = BATCH // NCORES        # 8192 batch rows per core
SC = 1024                    # batch rows per superchunk (relu granularity)
NSC = BPC // SC              # 8
CHUNK = 128                  # batch rows per mm2sel chunk (PE stationary width)
NCH = BPC // CHUNK           # 64 chunks per (group, core)
SEGS = 3                     # padded segments per chunk (fixed for SPMD)
ZC = NCH * SEGS * 2          # z cols per group = 384
W1SCALE = 16.0

FP8 = mybir.dt.float8e4
BF16 = mybir.dt.bfloat16
F32 = mybir.dt.float32
NP_FP8 = ml_dtypes.float8_e4m3
NP_BF16 = ml_dtypes.bfloat16

# group n takes columns n, n+8, ... (round-robin)
GROUP_IDX = np.stack([np.arange(n, OBS, G) for n in range(G)])  # (g, gs)


def build_program(num_devices: int = NCORES):
    """Per-core bass program (SPMD: identical across cores; per-core data
    differences live in xt / w2sel)."""
    nc = bacc.Bacc("TRN2", target_bir_lowering=False, debug=False,
                   num_devices=num_devices)

    # xt[g, p, sc, i, b] = Xg_sorted[sc*SC + b, p + 64*i]  (fp8)
    xt = nc.dram_tensor("xt", [G, 64, NSC, 2, SC], FP8, kind="ExternalInput").ap()
    # w1[p, g, i, m] = 16 * W1[g, p + 64*i, m]  (fp8)
    w1 = nc.dram_tensor("w1", [64, G, 2, HID], FP8, kind="ExternalInput").ap()
    # w2sel[k, g, ch, s, j] = W2[g, k, l(g,ch,s) + 64*j] / 16  (bf16)
    w2sel = nc.dram_tensor("w2sel", [HID, G, NCH, SEGS, 2], BF16,
                           kind="ExternalInput").ap()
    # b1s[k, g] = 16 * b1[g, k]
    b1 = nc.dram_tensor("b1", [HID, G], F32, kind="ExternalInput").ap()
    # zout[g][row, (ch*SEGS+s)*2 + j]: j=0 -> zm, j=1 -> zv  (bf16)
    zout = nc.dram_tensor("z", [G, CHUNK, ZC], BF16, kind="ExternalOutput").ap()

    # --- static engine load balancer for the vector ops -------------------
    # op cost model (ns) for [*, n]-col ops per engine; greedy least-loaded
    eng_time = {"act": 0.0, "dve": 0.0}

    def relu_cost(e, n):
        if e == "act":
            return n * 0.833 + 185.0
        return n * 1.042 + 125.0

    def pick_engine(n):
        e = min(eng_time, key=lambda k: eng_time[k] + relu_cost(k, n))
        eng_time[e] += relu_cost(e, n)
        return e

    from contextlib import ExitStack
    with tile.TileContext(nc) as tc, ExitStack() as st:
        cp = st.enter_context(tc.tile_pool(name="const", bufs=1))
        w1_sb = cp.tile([64, G, 2, HID], FP8, tag="w1")
        nc.sync.dma_start(w1_sb[:], w1)
        b1_sb = cp.tile([HID, G], F32, tag="b1")
        nc.sync.dma_start(b1_sb[:], b1)
        # pre-load the ACT function table while DMAs run (LoadActFuncSet is
        # ~1.3us and would otherwise serialize with the first relu)
        warm = cp.tile([1, 1], F32, tag="warm")
        nc.vector.memset(warm[:], 0.0)
        warm2 = cp.tile([1, 1], F32, tag="warm2")
        nc.scalar.activation(warm2[:], warm[:],
                             mybir.ActivationFunctionType.Relu,
                             bias=0.0, scale=1.0)

        xpool = st.enter_context(tc.tile_pool(name="xg", bufs=4))
        wspool = st.enter_context(tc.tile_pool(name="ws", bufs=2))
        hpool = st.enter_context(tc.tile_pool(name="hsb", bufs=10))
        # one zsb per group: a drain must NEVER wait on a zout DMA (those
        # queue behind xt transfers on the serialized DMA engines, and a
        # stalled drain blocks every later relu in its engine's in-order queue)
        zspool = st.enter_context(tc.tile_pool(name="zsb", bufs=8))
        hpsum = st.enter_context(tc.tile_pool(name="hp", bufs=3, space="PSUM"))
        zpsum = st.enter_context(tc.tile_pool(name="zt", bufs=2, space="PSUM"))

        relu_fns = {
            "act": lambda o, i, b: nc.scalar.activation(
                o, i, mybir.ActivationFunctionType.Relu, bias=b, scale=1.0),
            "dve": lambda o, i, b: nc.vector.tensor_scalar(
                o, i, b, 0.0, mybir.AluOpType.add, mybir.AluOpType.max),
        }
        copy_fns = {
            "act": nc.scalar.copy,
            "dve": nc.vector.tensor_copy,
        }


        # software-pipelined emission: PE sel-matmuls run one instance behind
        pending = []            # (g, sc, hsb, zt)
        gdone = []              # (g, zt) awaiting drain after last sel emitted

        def emit_sel(item):
            # one matmul per 128-batch chunk: moving = all SEGS*2 contiguous
            # W2 columns for that chunk (fewer PE instructions -> less queue
            # transit on the critical path)
            g, sc, hsb, zt = item
            w = SEGS * 2
            for cc in range(SC // CHUNK):
                ch = sc * (SC // CHUNK) + cc
                nc.tensor.matmul(
                    zt[:, ch * w:(ch + 1) * w],
                    hsb[:, CHUNK * cc:CHUNK * (cc + 1)],
                    wsel_tiles[g][:, g % 4, ch],
                    start=True, stop=True, skip_group_check=True)
            if sc == NSC - 1:
                gdone.append((g, zt))

        def emit_drain():
            g, zt = gdone.pop(0)
            e = pick_engine(ZC)
            zsb = zspool.tile([CHUNK, ZC], BF16, name=f"zsb{g}", tag="zsb")
            copy_fns[e](zsb[:], zt[:, :ZC])
            if g == G - 1:
                # tail latency matters: SP HWDGE is faster than SWDGE
                nc.sync.dma_start(zout[g], zsb[:])
            else:
                # SWDGE path: Pool engine is otherwise idle and bypasses the
                # shared HWDGE slot.
                nc.gpsimd.dma_start(zout[g], zsb[:])

        wsel_tiles = {}
        for g in range(G):
            # prefetch xt for group g as two half-DMAs (amortizes the ~625ns
            # HWDGE fixed cost while keeping startup latency low)
            xg = xpool.tile([64, NSC, 2, SC], FP8, name=f"xg{g}", tag="xg")
            nparts = 4 if g == 0 else 2   # finer first DMA -> earlier start
            psc = NSC // nparts
            for h in range(nparts):
                nc.sync.dma_start(xg[:, h * psc:(h + 1) * psc],
                                  xt[g, :, h * psc:(h + 1) * psc])
            if g == 0:
                # both wsel DMAs upfront: emitting ws1 at g=4 would queue it
                # on SP behind slot-blocked xt DMAs, starving g>=4 sels
                for wh in (0, 1):
                    wsel = wspool.tile([HID, 4, NCH, SEGS, 2], BF16,
                                       name=f"ws{wh}", tag="wsel")
                    nc.sync.dma_start(wsel[:], w2sel[:, 4 * wh:4 * wh + 4])
                    for gg in range(4 * wh, 4 * wh + 4):
                        wsel_tiles[gg] = wsel
            zt = zpsum.tile([CHUNK, 512], F32, name=f"zt{g}", tag="zt")
            for sc in range(NSC):
                hp = hpsum.tile([HID, SC], F32, tag="hp")
                for half in range(SC // 512):
                    nc.tensor.matmul(
                        hp[:, 512 * half:512 * (half + 1)],
                        w1_sb[:, g],
                        xg[:, sc, :, 512 * half:512 * (half + 1)],
                        start=True, stop=True,
                        perf_mode=mybir.MatmulPerfMode.DoubleRow)
                hsb = hpool.tile([HID, SC], BF16, tag="hsb")
                e = pick_engine(SC)
                relu_fns[e](hsb[:], hp[:], b1_sb[:, g:g + 1])

                pending.append((g, sc, hsb, zt))
                # skew: keep sel-matmuls (which wait on relu i) from
                # head-of-line-blocking later mm1s in the in-order PE queue
                if len(pending) > 5:
                    emit_sel(pending.pop(0))
                # drain-skew: emit drains well after the group's last sels so
                # the drain never parks in ACT/DVE's in-order queue waiting
                if len(gdone) > 0 and (sc >= 3 or gdone[0][0] == g - 2):
                    emit_drain()
        while pending:
            emit_sel(pending.pop(0))
        while gdone:
            emit_drain()

    nc.compile()
    return nc


# ---------------------------------------------------------------- host side --

def _prep_host(X, eps, W1, b1, W2, b2, indices, ncores=NCORES):
    """Per-core input dicts + metadata for unscrambling."""
    W1p = np.ascontiguousarray(
        (W1 * W1SCALE).reshape(G, 2, 64, HID).transpose(2, 0, 1, 3)
    ).astype(NP_FP8)                                   # (64, G, 2, HID)
    b1s = np.ascontiguousarray((W1SCALE * b1).T).astype(np.float32)  # (HID, G)
    W2s = (W2 / W1SCALE).astype(np.float32)            # (G, HID, 128)

    in_maps = []
    metas = []
    for core in range(ncores):
        lo = core * BPC
        xt = np.empty((G, 64, NSC, 2, SC), NP_FP8)
        w2sel = np.empty((HID, G, NCH, SEGS, 2), NP_BF16)
        meta = []
        for g in range(G):
            idxg = indices[g, lo:lo + BPC]
            order = np.argsort(idxg, kind="stable")    # sorted batch positions
            slat = idxg[order]                         # (BPC,) sorted latents
            Xg = X[lo + order][:, GROUP_IDX[g]].astype(NP_FP8)  # (BPC, 128)
            # pack [p, sc, i, b]: col k = p + 64*i
            xt[g] = (Xg.reshape(NSC, SC, 2, 64)
                     .transpose(3, 0, 2, 1))           # (64, NSC, 2, SC)
            # segments: distinct latents per 128-chunk, padded to SEGS
            lat_ch = slat.reshape(NCH, CHUNK)
            seg_lat = np.zeros((NCH, SEGS), np.int64)
            seg_of_pos = np.empty(BPC, np.int64)
            for ch in range(NCH):
                uniq, inv = np.unique(lat_ch[ch], return_inverse=True)
                ns = len(uniq)
                assert ns <= SEGS, f"chunk needs {ns} segments > SEGS={SEGS}"
                seg_lat[ch, :ns] = uniq
                seg_of_pos[ch * CHUNK:(ch + 1) * CHUNK] = inv
            # w2sel[k, ch, s, j] = W2s[g][k, seg_lat[ch,s] + 64*j]
            cols = (seg_lat[None, :, :, None] +
                    64 * np.arange(2)[None, None, None, :])  # (1, NCH, SEGS, 2)
            w2sel[:, g] = W2s[g][:, cols[0]].astype(NP_BF16)
            meta.append((order, slat, seg_of_pos))
        in_maps.append({"xt": xt, "w1": W1p, "w2sel": w2sel, "b1": b1s})
        metas.append(meta)
    return in_maps, metas


def _finish_host(zdev, meta, eps_c, b2):
    """zdev: (G, CHUNK, ZC) f32; returns z (G, BPC) in original batch order."""
    z = np.empty((G, BPC), np.float32)
    pos = np.arange(BPC)
    rows = pos % CHUNK
    ch = pos // CHUNK
    for g in range(G):
        order, slat, seg_of_pos = meta[g]
        col = (ch * SEGS + seg_of_pos) * 2
        zm = zdev[g][rows, col]
        zv = zdev[g][rows, col + 1]
        zs = (zm + b2[g, slat] +
              eps_c[g, order] * np.exp(0.5 * (zv + b2[g, LAT + slat])))
        z[g, order] = zs
    return z


_NC_CACHE = {}


def kernel(X, eps, W1, b1, W2, b2, indices):
    if "nc" not in _NC_CACHE:
        _NC_CACHE["nc"] = build_program(NCORES)
    nc = _NC_CACHE["nc"]
    in_maps, metas = _prep_host(X, eps, W1, b1, W2, b2, indices)
    res = bass_utils.run_bass_kernel_spmd(nc, in_maps,
                                          core_ids=list(range(NCORES)))
    z = np.zeros((G, BATCH), np.float32)
    for core in range(NCORES):
        lo = core * BPC
        zdev = np.asarray(res.results[core]["z"]).astype(np.float32)
        z[:, lo:lo + BPC] = _finish_host(zdev, metas[core],
                                         eps[:, lo:lo + BPC], b2)
    return z.astype(np.float32)
